# revision 6
# baseline (speedup 1.0000x reference)
"""Trainium2 Bass kernel for the gnn_message_passing problem.

Contract: kernel(**inputs) takes the FULL unsharded inputs (numpy, keyed as in
setup_inputs()) and returns the FULL output [16, 32, 100, 1024] float32.

Strategy: pure data parallel over batch*time (BT = 512 graphs) across 8
NeuronCores (64 graphs each). All math runs on device; the host only does
layout packing (transpose/cast/shard) and unpacking.

Per graph g (lf [100, 1024], gf [49, 1024]):
  rl[n] = 1/||lf[n]||, rg[m] = 1/||gf[m]||      (squares + ones-matmul)
  A_raw = (lf @ gf^T) * outer(rl, rg)           (norms folded after matmul)
  A     = softmax(5 * A_raw, axis=-1)           (Exp with fused row-sum)
  y     = [A | A_raw | 1] @ [gf@W1^T ; W2^T ; b]  (stacked matmul, k <= 114)
  out   = LeakyReLU(LayerNorm(y))               (bn_stats + fused Lrelu pass)

gf@W1^T is computed for a PAIR of graphs in one matmul chain (stationary
[128, 2*64] with the m-dim zero-padded to 64 so each graph's slab sits at a
32-aligned partition offset). Odd graphs use a reordered stack so their gfW1
slab (psum partitions 64:113) lands in the rhs stack without any cross-
partition move.

W_adj is applied honestly when it is not the identity (general path computes
Q = W_adj @ gf^T first); when it is exactly identity the application is a
numerical no-op and is skipped. Same for ln_g == 1 / ln_b == 0.
"""

import numpy as np
import ml_dtypes

B, T, N, C = 16, 32, 100, 1024
M = 49
MP = 64  # m padded to a 32-aligned slab
BT = B * T
NCORES = 8
GPC = BT // NCORES  # graphs per core
QPC = GPC // 2  # graph pairs per core
CT = C // 128  # contraction tiles

_BF16 = ml_dtypes.bfloat16


def _build(general_w: bool, general_ln: bool):
    import concourse.bacc as bacc
    import concourse.mybir as mybir
    import concourse.tile as tile
    from concourse import masks

    AF = mybir.ActivationFunctionType
    ALU = mybir.AluOpType
    bf16 = mybir.dt.bfloat16
    f32 = mybir.dt.float32

    nc = bacc.Bacc("TRN2", target_bir_lowering=False, debug=False,
                   num_devices=NCORES)

    lft = nc.dram_tensor("lft", [GPC, 128, CT, N], bf16, kind="ExternalInput")
    gfp = nc.dram_tensor("gfp", [QPC, 128, CT, 2, MP], bf16,
                         kind="ExternalInput")
    w1t = nc.dram_tensor("w1t", [128, CT, C], bf16, kind="ExternalInput")
    w2tb = nc.dram_tensor("w2tb", [M + 1, C], bf16, kind="ExternalInput")
    if general_w:
        wadjt = nc.dram_tensor("wadjt", [128, CT, CT, 128], bf16,
                               kind="ExternalInput")
    if general_ln:
        grep = nc.dram_tensor("grep", [128, C], f32, kind="ExternalInput")
        brep = nc.dram_tensor("brep", [128, C], f32, kind="ExternalInput")
    out = nc.dram_tensor("out", [GPC, N, C], f32, kind="ExternalOutput")

    # per-j geometry of the stacked matmul
    #  j=0: rhs rows = [gfW1(0:49) | W2T(49:98) | b(98)]          k = 99
    #  j=1: rhs rows = [W2T(0:49) | b(49) | 0(50:64) | gfW1(64:113)]  k = 114
    KJ = [2 * M + 1, MP + M]
    ANORM_COL = [0, MP]
    ARAW_COL = [M, 0]
    ONES_COL = [2 * M, M]

    with tile.TileContext(nc) as tc:
        with (
            tc.tile_pool(name="statics", bufs=1) as statics,
            tc.tile_pool(name="pair_sb", bufs=2) as pair_sb,
            tc.tile_pool(name="graph_sb", bufs=3) as graph_sb,
            tc.tile_pool(name="ps_small", bufs=2, space="PSUM") as ps_small,
            tc.tile_pool(name="ps_pair", bufs=1, space="PSUM") as ps_pair,
            tc.tile_pool(name="ps_y", bufs=1 if general_w else 2,
                         space="PSUM") as ps_y,
        ):
            # ---- static tiles ----
            ident = statics.tile([128, 128], f32)
            masks.make_identity(nc, ident[:])
            onecol = statics.tile([128, 1], bf16)
            nc.gpsimd.memset(onecol[:], 1.0)
            epsln = statics.tile([128, 1], f32)
            nc.gpsimd.memset(epsln[:], 1e-5)
            w1t_sb = statics.tile([128, CT, C], bf16)
            nc.sync.dma_start(w1t_sb[:], w1t.ap())
            # persistent R-stacks (static rows loaded once; gfW1 rows per graph)
            rstk0 = statics.tile([2 * M + 1, C], bf16)
            nc.sync.dma_start(rstk0[M:2 * M + 1, :], w2tb.ap())
            rstk1 = statics.tile([MP + M, C], bf16)
            nc.gpsimd.memset(rstk1[0:MP, :], 0.0)
            nc.sync.dma_start(rstk1[0:M, :], w2tb.ap()[0:M])
            nc.sync.dma_start(rstk1[M:M + 1, :], w2tb.ap()[M:M + 1])
            rstk = [rstk0, rstk1]
            if general_w:
                wadj_sb = statics.tile([128, CT, CT, 128], bf16)
                nc.sync.dma_start(wadj_sb[:], wadjt.ap())
            if general_ln:
                grep_sb = statics.tile([128, C], f32)
                brep_sb = statics.tile([128, C], f32)
                nc.sync.dma_start(grep_sb[:], grep.ap())
                nc.sync.dma_start(brep_sb[:], brep.ap())

            for q in range(QPC):
                # ---- per-pair work ----
                gfp_t = pair_sb.tile([128, CT, 2, MP], bf16, tag="gfp")
                nc.sync.dma_start(gfp_t[:], gfp.ap()[q])

                # gfW1 = gf @ W1^T for both graphs: psum [128, 1024],
                # graph 0 at partitions 0:49, graph 1 at partitions 64:113
                pw = ps_pair.tile([128, C], f32, tag="pw")
                for ct in range(CT):
                    for h in range(2):
                        nc.tensor.matmul(
                            pw[:, h * 512:(h + 1) * 512],
                            gfp_t[:, ct, :, :],
                            w1t_sb[:, ct, h * 512:(h + 1) * 512],
                            start=(ct == 0), stop=(ct == CT - 1),
                        )

                if general_w:
                    # Q = W_adj @ gf^T for both graphs: psum [128, CT*2*MP]
                    qps = ps_pair.tile([128, CT, 2, MP], f32, tag="qps")
                    for dt_i in range(CT):
                        for ct in range(CT):
                            nc.tensor.matmul(
                                qps[:, dt_i, :, :],
                                wadj_sb[:, ct, dt_i, :],
                                gfp_t[:, ct, :, :],
                                start=(ct == 0), stop=(ct == CT - 1),
                            )
                    qp_sb = pair_sb.tile([128, CT, 2, MP], bf16, tag="qp")
                    nc.scalar.activation(qp_sb[:], qps[:], AF.Copy)
                    rhs_pm = qp_sb
                else:
                    rhs_pm = gfp_t

                # rg = 1/||gf_m|| for both graphs (row layout [1, 2*MP])
                sqg = pair_sb.tile([128, CT, 2, MP], bf16, tag="sqg")
                nc.vector.tensor_tensor(
                    out=sqg[:], in0=gfp_t[:], in1=gfp_t[:], op=ALU.mult)
                rg_ps = ps_small.tile([128, 512], f32, tag="sm")
                for ct in range(CT):
                    nc.tensor.matmul(
                        rg_ps[0:1, 0:2 * MP], onecol[:], sqg[:, ct, :, :],
                        start=(ct == 0), stop=(ct == CT - 1))
                rg_f = pair_sb.tile([1, 2, MP], f32, tag="rgf")
                nc.vector.reciprocal(
                    rg_f[:, 0, 0:M], rg_ps[0:1, 0:M])
                nc.vector.reciprocal(
                    rg_f[:, 1, 0:M], rg_ps[0:1, MP:MP + M])
                rg_row = pair_sb.tile([1, 2, MP], bf16, tag="rgr")
                nc.scalar.activation(rg_row[:, 0, 0:M], rg_f[:, 0, 0:M],
                                     AF.Sqrt)
                nc.scalar.activation(rg_row[:, 1, 0:M], rg_f[:, 1, 0:M],
                                     AF.Sqrt)

                for j in range(2):
                    g = 2 * q + j
                    kj = KJ[j]
                    # ---- per-graph work ----
                    lft_t = graph_sb.tile([128, CT, N], bf16, tag="lft")
                    nc.sync.dma_start(lft_t[:], lft.ap()[g])

                    # rl = 1/||lf_n|| (row layout [1, 100])
                    sql = graph_sb.tile([128, CT, N], bf16, tag="sql")
                    nc.vector.tensor_tensor(
                        out=sql[:], in0=lft_t[:], in1=lft_t[:], op=ALU.mult)
                    sm = ps_small.tile([128, 512], f32, tag="sm")
                    for ct in range(CT):
                        nc.tensor.matmul(
                            sm[0:1, 256:256 + N], onecol[:], sql[:, ct, :],
                            start=(ct == 0), stop=(ct == CT - 1))
                    sl_f = graph_sb.tile([1, N], f32, tag="slf")
                    nc.vector.reciprocal(sl_f[:], sm[0:1, 256:256 + N])
                    rl_row = graph_sb.tile([1, N], bf16, tag="rlr")
                    nc.scalar.activation(rl_row[:], sl_f[:], AF.Sqrt)

                    # S = outer(rl, rg_j): psum [100, 49] at cols 64:113
                    nc.tensor.matmul(
                        sm[0:N, 64:64 + M], rl_row[:],
                        rg_row[:, j, 0:M],
                        start=True, stop=True)
                    s_sb = graph_sb.tile([N, M], f32, tag="s_sb")
                    nc.scalar.activation(s_sb[:], sm[0:N, 64:64 + M], AF.Copy)

                    # P_raw = lf @ gf^T (or lf @ Q^T): psum [100, 49] cols 0:49
                    for ct in range(CT):
                        nc.tensor.matmul(
                            sm[0:N, 0:M], lft_t[:, ct, :],
                            rhs_pm[:, ct, j, 0:M],
                            start=(ct == 0), stop=(ct == CT - 1))

                    # stack columns (f32): A_norm | A_raw | ones per KJ[j]
                    stack = graph_sb.tile([N, 128], f32, tag="stack")
                    araw = stack[:, ARAW_COL[j]:ARAW_COL[j] + M]
                    nc.vector.tensor_tensor(
                        out=araw, in0=sm[0:N, 0:M], in1=s_sb[:], op=ALU.mult)
                    nc.gpsimd.memset(
                        stack[:, ONES_COL[j]:ONES_COL[j] + 1], 1.0)
                    if j == 1:
                        nc.gpsimd.memset(stack[:, M + 1:MP, ], 0.0)

                    e_t = graph_sb.tile([N, M], f32, tag="e")
                    ssum = graph_sb.tile([N, 1], f32, tag="ssum")
                    nc.scalar.activation(
                        e_t[:], araw, AF.Exp, scale=5.0, accum_out=ssum[:])
                    sinv = graph_sb.tile([N, 1], f32, tag="sinv")
                    nc.vector.reciprocal(sinv[:], ssum[:])
                    nc.vector.tensor_scalar(
                        out=stack[:, ANORM_COL[j]:ANORM_COL[j] + M],
                        in0=e_t[:], scalar1=sinv[:],
                        scalar2=None, op0=ALU.mult)

                    # transpose stack -> [kj, 100] psum cols 128:228
                    nc.tensor.transpose(
                        sm[0:kj, 128:128 + N], stack[:, 0:kj],
                        ident[0:N, 0:N])
                    lhs_y = graph_sb.tile([128, N], bf16, tag="lhy")
                    nc.scalar.activation(
                        lhs_y[0:kj, :], sm[0:kj, 128:128 + N], AF.Copy)

                    # R-stack gfW1 rows <- pw slab (no partition shift)
                    if j == 0:
                        nc.scalar.activation(
                            rstk0[0:M, :], pw[0:M, :], AF.Copy)
                    else:
                        nc.scalar.activation(
                            rstk1[MP:MP + M, :], pw[MP:MP + M, :], AF.Copy)

                    # y = stack^T.T @ Rstack : psum [100, 1024]
                    yps = ps_y.tile([N, C], f32, tag="y")
                    for h in range(2):
                        nc.tensor.matmul(
                            yps[:, h * 512:(h + 1) * 512], lhs_y[0:kj, :],
                            rstk[j][:, h * 512:(h + 1) * 512],
                            start=True, stop=True)

                    # LayerNorm stats
                    stats = graph_sb.tile([N, 2, 6], f32, tag="stats")
                    yps_v = yps[:].rearrange("p (a b) -> p a b", a=2)
                    nc.vector.bn_stats(out=stats[:, 0, :], in_=yps_v[:, 0, :])
                    nc.vector.bn_stats(out=stats[:, 1, :], in_=yps_v[:, 1, :])
                    mv = graph_sb.tile([N, 2], f32, tag="mv")
                    nc.vector.bn_aggr(out=mv[:], in_=stats[:])
                    rstd = graph_sb.tile([N, 1], f32, tag="rstd")
                    nc.scalar.activation(
                        rstd[:], mv[:, 1:2], AF.Sqrt, bias=epsln[0:N])
                    nc.vector.reciprocal(rstd[:], rstd[:])
                    negmurs = graph_sb.tile([N, 1], f32, tag="negmurs")
                    nc.vector.tensor_scalar(
                        out=negmurs[:], in0=mv[:, 0:1], scalar1=rstd[:],
                        scalar2=-1.0, op0=ALU.mult, op1=ALU.mult)

                    y_out = graph_sb.tile([N, C], f32, tag="yo")
                    if general_ln:
                        nc.scalar.activation(
                            y_out[:], yps[:], AF.Copy, bias=negmurs[:],
                            scale=rstd[:])
                        nc.vector.tensor_tensor(
                            out=y_out[:], in0=y_out[:], in1=grep_sb[0:N, :],
                            op=ALU.mult)
                        nc.vector.tensor_tensor(
                            out=y_out[:], in0=y_out[:], in1=brep_sb[0:N, :],
                            op=ALU.add)
                        nc.scalar.activation(
                            y_out[:], y_out[:], AF.Lrelu, alpha=0.01)
                    else:
                        nc.scalar.activation(
                            y_out[:], yps[:], AF.Lrelu, bias=negmurs[:],
                            scale=rstd[:], alpha=0.01)
                    nc.sync.dma_start(out.ap()[g], y_out[:])

    nc.compile()
    return nc


_cache = {}


def _get_nc(general_w: bool, general_ln: bool):
    key = (general_w, general_ln)
    if key not in _cache:
        _cache[key] = _build(general_w, general_ln)
    return _cache[key]


def _pack_inputs(local_feat, global_feat, W_aff, b_aff):
    lf = np.ascontiguousarray(local_feat.reshape(BT, N, C))
    gf = np.ascontiguousarray(global_feat.reshape(BT, M, C))
    # lft[g, p, t, n] = lf[g, n, t*128+p]
    lft = lf.transpose(0, 2, 1).reshape(BT, CT, 128, N).transpose(0, 2, 1, 3)
    lft = np.ascontiguousarray(lft.astype(_BF16))
    # gfp[q, p, t, j, m] = gf[2q+j, m, t*128+p], m zero-padded 49 -> 64
    gfp = np.zeros((BT // 2, 128, CT, 2, MP), dtype=_BF16)
    g4 = gf.transpose(0, 2, 1).reshape(BT // 2, 2, CT, 128, M)
    gfp[:, :, :, :, 0:M] = g4.transpose(0, 3, 2, 1, 4).astype(_BF16)
    # w1t[p, t, co] = W_aff[co, t*128+p]
    w1t = np.ascontiguousarray(
        W_aff[:, :C].T.reshape(CT, 128, C).transpose(1, 0, 2).astype(_BF16))
    # w2tb rows 0:49 = W2^T, row 49 = b_aff
    w2tb = np.concatenate([W_aff[:, C:C + M].T, b_aff[None, :]], axis=0)
    w2tb = np.ascontiguousarray(w2tb.astype(_BF16))
    return lft, gfp, w1t, w2tb


def _make_in_maps(lft, gfp, w1t, w2tb, extra):
    shared = {"w1t": w1t, "w2tb": w2tb, **extra}
    in_maps = []
    for k in range(NCORES):
        gs = slice(k * GPC, (k + 1) * GPC)
        qs = slice(k * QPC, (k + 1) * QPC)
        in_maps.append({"lft": np.ascontiguousarray(lft[gs]),
                        "gfp": np.ascontiguousarray(gfp[qs]), **shared})
    return in_maps


def kernel(local_feat, global_feat, pos, W_adj, W_aff, b_aff, ln_g, ln_b):
    from concourse.bass_utils import run_bass_kernel_spmd

    general_w = not np.array_equal(W_adj, np.eye(C, dtype=W_adj.dtype))
    general_ln = not (np.all(ln_g == 1.0) and np.all(ln_b == 0.0))

    lft, gfp, w1t, w2tb = _pack_inputs(local_feat, global_feat, W_aff, b_aff)

    extra = {}
    if general_w:
        # wadjt[p, ct, dt, d] = W_adj[dt*128+d, ct*128+p]
        wadjt = W_adj.T.reshape(CT, 128, CT, 128).transpose(1, 0, 2, 3)
        extra["wadjt"] = np.ascontiguousarray(wadjt.astype(_BF16))
    if general_ln:
        extra["grep"] = np.ascontiguousarray(
            np.broadcast_to(ln_g[None, :], (128, C)).astype(np.float32))
        extra["brep"] = np.ascontiguousarray(
            np.broadcast_to(ln_b[None, :], (128, C)).astype(np.float32))

    nc = _get_nc(general_w, general_ln)
    in_maps = _make_in_maps(lft, gfp, w1t, w2tb, extra)

    res = run_bass_kernel_spmd(nc, in_maps, core_ids=list(range(NCORES)))
    y = np.concatenate([res.results[k]["out"] for k in range(NCORES)], axis=0)
    return np.ascontiguousarray(y.reshape(B, T, N, C).astype(np.float32))
